# revision 17
# baseline (speedup 1.0000x reference)
"""Distributed multi-head attention block on 8 TRN2 NeuronCores.

Reference computation (B=2, S=2048, D=1024, H=16, DH=64):
    q = split_heads(q_ @ Wq + bq); k = ...; v = ...
    attn = softmax(q k^T / 8)  (mask is all-ones -> identity row mask)
    out = (merge_heads(attn @ v) + q_) @ Wf + bf

Sharding: 16 heads split 8 ways (2 heads / core); each core handles BOTH
batches.  The "virtual q" axis is b-major: vq = b*2048 + s (4096 total).

Per core c (heads 2c, 2c+1; d-dims 128c..128c+128):
  1. Projections (fp8, DoubleRow over din-tile pairs): QT/KT [128 dh,
     4096 vq], V [vk, 130] tiles ([h0 64 | 1 | h1 64 | 1] -- the ones
     columns make the PV matmul emit softmax denominators for free).
     Q/K inputs are host-tiled per (batch, q-chunk) so each 512-token
     chunk is one contiguous DMA and its projection starts as soon as
     that chunk lands.
  2. Attention, transposed formulation: ST[k, q] = KT^T Q (bf16, two
     64-row head tiles run concurrently); exp straight from PSUM into
     fp8 pair-tiles, split between ScalarE (exact ACT) and VectorE
     (Schraudolph bit-trick: uint8(st*a+b) reinterpreted as fp8e4m3 --
     RNE saturating convert makes this as accurate as exact-exp+fp8
     rounding); OT[65, q] = [V|1]^T P as fp8 DoubleRow over k-tile
     pairs (row 64 = denominator); normalize via approx-reciprocal
     (VectorE) + multiply (GpSimd) -> ZT_local [128 d, 4096 vq] (fp8e5).
  3. One 8-core AllToAll exchanges q-slices.  The final fc is split:
     Y1 = (WS*xres)^T @ (Wf/WS) accumulates into PSUM while the
     collective runs, then Y2 = zt^T @ (Wf/WS) (fp8e5 DoubleRow)
     accumulates on top once the exchange lands.

Host side: casts/transposes/pre-tiles inputs (numpy), feeds per-core
shards, places each core's [512, 1024] output chunk, adds bf.  If the mask
is not all-ones (never happens with this problem's generator), falls back
to a numpy reference implementation.
"""

import math
import sys

sys.path.insert(0, "/opt/trn_rl_repo")

import ml_dtypes
import numpy as np

import concourse.bass as bass
import concourse.tile as tile
from concourse import bacc, mybir
from concourse.bass_utils import run_bass_kernel_spmd

B, S, D, H = 2, 2048, 1024, 16
DH = D // H  # 64
N_CORES = 8
VQ = B * S  # 4096 virtual q (b-major)
NQC = VQ // 512  # 8 q-chunks of 512
NKT = S // 128  # 16 k-tiles per batch
NDIN = D // 128  # 8 din tiles

BF16 = mybir.dt.bfloat16
FP8 = mybir.dt.float8e4
FP8E5 = mybir.dt.float8e5
U8 = mybir.dt.uint8
F32 = mybir.dt.float32
AF = mybir.ActivationFunctionType
ALU = mybir.AluOpType
PM = mybir.MatmulPerfMode
BF16NP = ml_dtypes.bfloat16
FP8NP = ml_dtypes.float8_e4m3
FP8E5NP = ml_dtypes.float8_e5m2
WSCALE = 32.0

# Schraudolph constants: t = st*EXPA + EXPB; uint8(t) bits == fp8e4m3 of
# ~exp(st*0.125/WSCALE^2 - 3).  -0.458 centers the piecewise-linear
# log2 approximation (mean-neutral shift).
EXP_SCALE = 0.125 / (WSCALE * WSCALE)
EXPA = EXP_SCALE * 8.0 / math.log(2.0)
EXPB = -3.0 * 8.0 / math.log(2.0) + 56.0 - 0.458

WARMUP_MM = 130  # cover the xq0c0 DMA wait (~13us)
KEEPWARM_MM = 72  # cover the AllToAll window

_CACHE = {}

QCW = NDIN * 512  # 4096 cols per (b, qc) input chunk
VTW = 144  # v tile width: [h0 64 | 1 | h1 64 | 1 | pad 14] (DR step must be %16)


def _use_dve(qc, kt):
    # ~5/16 of exp tiles go to VectorE (it also carries the normalize
    # copies); the rest to ScalarE.
    if kt == 0 and (qc & 1) == 0:
        return False
    return (2 * kt + (qc & 1)) % 3 == 0


def _build():
    nc = bacc.Bacc(None, target_bir_lowering=False)

    # xq/xk host-pre-tiled per (b, qc): [128, (b qc t v)] with v=512
    xq = nc.declare_dram_parameter("xq", [128, B * 4 * QCW], FP8, isOutput=False)
    xk = nc.declare_dram_parameter("xk", [128, B * 4 * QCW], FP8, isOutput=False)
    # xv tiled per batch: [128, (b t v)] with v=2048
    xv = nc.declare_dram_parameter("xv", [128, B * NDIN * 2048], FP8, isOutput=False)
    wq = nc.declare_dram_parameter("wq", [128, NDIN * 128], FP8, isOutput=False)
    wk = nc.declare_dram_parameter("wk", [128, NDIN * 128], FP8, isOutput=False)
    wv = nc.declare_dram_parameter("wv", [128, NDIN * VTW], FP8, isOutput=False)
    wf = nc.declare_dram_parameter("wf", [128, NDIN * 1024], BF16, isOutput=False)
    wf8 = nc.declare_dram_parameter("wf8", [128, NDIN * 1024], FP8E5, isOutput=False)
    xresfc = nc.declare_dram_parameter("xresfc", [128, NDIN * 512], BF16, isOutput=False)
    bq = nc.declare_dram_parameter("bq", [128, 1], F32, isOutput=False)
    bk = nc.declare_dram_parameter("bk", [128, 1], F32, isOutput=False)
    bvx = nc.declare_dram_parameter("bvx", [1, VTW], BF16, isOutput=False)
    out = nc.declare_dram_parameter("out", [512, D], F32, isOutput=True)

    with tile.TileContext(nc) as tc:
        with (
            tc.tile_pool(name="persist", bufs=1) as sbp,
            tc.tile_pool(name="dram", bufs=1, space="DRAM") as dram,
        ):
            # ---- persistent SBUF tensors ----
            qt_sb = sbp.tile([128, VQ], BF16)  # [2 heads x 64 dh, vq]
            kt_sb = sbp.tile([128, VQ], BF16)  # [2 heads x 64 dh, vkey]
            # V: per (b,kt) tile of 130 cols: [h0 64 | 1 | h1 64 | 1]
            v_sb = sbp.tile([128, 32 * VTW], FP8)
            wq_sb = sbp.tile([128, NDIN * 128], FP8)
            wk_sb = sbp.tile([128, NDIN * 128], FP8)
            wv_sb = sbp.tile([128, NDIN * VTW], FP8)
            wf_sb = sbp.tile([128, NDIN * 1024], BF16)
            wf8_sb = sbp.tile([128, NDIN * 1024], FP8E5)
            xresfc_sb = sbp.tile([128, NQC * 512], BF16)
            zt_local = sbp.tile([128, VQ], FP8E5)
            bq_sb = sbp.tile([128, 1], F32)
            bk_sb = sbp.tile([128, 1], F32)
            bvx_sb = sbp.tile([1, VTW], BF16)
            ones_sb = sbp.tile([128, 128], BF16)
            exp_warm = sbp.tile([1, 8], F32)
            nbias = sbp.tile([128, 1], F32)
            nc.vector.memset(ones_sb[:], 1.0)
            nc.vector.memset(nbias[:], -3.0)

            # ---- input DMAs ----
            xvp = tc.alloc_tile_pool(name="xvp", bufs=2)
            xin = tc.alloc_tile_pool(name="xin", bufs=2)
            xv_b = []
            xq_b = []
            xk_b = []
            for b in range(2):
                xv_b.append(xvp.tile([128, NDIN * 2048], FP8, name=f"xv{b}", tag="xv"))
                xq_b.append(xin.tile([128, 4 * QCW], FP8, name=f"xq{b}", tag="xq"))
                xk_b.append(xin.tile([128, 4 * QCW], FP8, name=f"xk{b}", tag="xk"))

            W = NDIN * 2048  # one xv batch-slab width

            def qk_chunk_dma(xt, xsrc, b, qc, eng=None):
                c0 = (4 * b + qc) * QCW
                (eng or nc.sync).dma_start(
                    xt[:, QCW * qc : QCW * (qc + 1)], xsrc[:, c0 : c0 + QCW]
                )

            nc.scalar.dma_start(bq_sb[:], bq[:])
            nc.scalar.dma_start(bk_sb[:], bk[:])
            nc.scalar.dma_start(wq_sb[:], wq[:])
            nc.scalar.dma_start(wk_sb[:], wk[:])
            # b0 inputs split across two DMA queues so they land ~2x sooner
            qk_chunk_dma(xq_b[0], xq, 0, 0)
            for qc_ in range(4):
                qk_chunk_dma(xk_b[0], xk, 0, qc_, eng=nc.scalar)
            for qc_ in range(1, 4):
                qk_chunk_dma(xq_b[0], xq, 0, qc_)
            nc.scalar.dma_start(wv_sb[:], wv[:])
            nc.scalar.dma_start(bvx_sb[:], bvx[:])
            nc.gpsimd.dma_start(xv_b[0][:], xv[:, 0:W])

            # preload the exp table set while the PE warms up
            nc.scalar.activation(exp_warm[:], exp_warm[:], AF.Exp, scale=1.0)

            def qk_proj_chunk(pool, name, xt, w_sb, b_sb, dst, b, qc):
                for u in qk_proj_subunits(pool, name, xt, w_sb, b_sb, dst, b, qc):
                    u()

            def qk_proj_subunits(pool, name, xt, w_sb, b_sb, dst, b, qc):
                # one 512-col projection chunk, fp8 DoubleRow over din-tile
                # pairs, split into 2 emission units
                pt = pool.tile([128, 512], F32, name=f"{name}c{b}_{qc}", tag="ps")
                xr = xt[:, QCW * qc : QCW * (qc + 1)].rearrange(
                    "p (t v) -> p t v", t=NDIN
                )
                wr = w_sb[:].rearrange("p (t n) -> p t n", t=NDIN)

                def half(h):
                    for dp in range(2 * h, 2 * h + 2):
                        nc.tensor.matmul(
                            pt[:],
                            lhsT=wr[:, 2 * dp : 2 * dp + 2, :],
                            rhs=xr[:, 2 * dp : 2 * dp + 2, :],
                            start=(dp == 0),
                            stop=(dp == NDIN // 2 - 1),
                            perf_mode=PM.DoubleRow,
                        )
                    if h == 1:
                        nc.vector.tensor_scalar_add(
                            dst[:, 512 * (4 * b + qc) : 512 * (4 * b + qc + 1)],
                            pt[:],
                            b_sb[:],
                        )

                return [lambda h=h: half(h) for h in range(2)]

            def v_subunits(pool, b, kt):
                # V projection for one (b, kt): 4 fp8 DoubleRow matmuls over
                # din-tile pairs streaming N=144 (the padded [h0|0|h1|0]
                # weight tile), a bias matmul whose rhs carries 1.0 at cols
                # 64/129 (fills the ones columns via has_written-overwrite),
                # and one contiguous [128,144] PSUM->SBUF copy.
                vkt = 16 * b + kt
                vp = pool.tile([128, 512], F32, name=f"vps{vkt}", tag="ps")
                xr = xv_b[b][:].rearrange("p (t v) -> p t v", t=NDIN)
                wr = wv_sb[:].rearrange("p (t n) -> p t n", t=NDIN)

                def half(t):
                    for dp in range(2 * t, 2 * t + 2):
                        nc.tensor.matmul(
                            vp[:, 0:VTW],
                            lhsT=xr[:, 2 * dp : 2 * dp + 2, 128 * kt : 128 * (kt + 1)],
                            rhs=wr[:, 2 * dp : 2 * dp + 2, :],
                            start=(dp == 0),
                            stop=False,
                            perf_mode=PM.DoubleRow,
                        )
                    if t == 1:
                        nc.tensor.matmul(
                            vp[:, 0:VTW], lhsT=ones_sb[0:1, :], rhs=bvx_sb[:],
                            start=False, stop=True,
                        )
                        nc.vector.tensor_copy(
                            v_sb[:, VTW * vkt : VTW * (vkt + 1)], vp[:, 0:VTW]
                        )

                return [lambda t=t: half(t) for t in range(2)]

            def v_unit(pool, b, kt):
                for u in v_subunits(pool, b, kt):
                    u()

            # ============ phase 1: warmup + Q0/K0 projections ============
            with tc.tile_pool(name="ps1a", bufs=4, space="PSUM") as ps1a:
                # K=1/M=1 dummies keep the HAM activity window busy (warm
                # clock) at ~1/16000th the power of full-array matmuls --
                # dense full-array dummies trip the GPIO power throttle
                warm = ps1a.tile([128, 512], F32, name="warm", tag="warm")
                for i in range(WARMUP_MM):
                    nc.tensor.matmul(
                        warm[0:1, 0:128], lhsT=ones_sb[0:1, 0:1],
                        rhs=ones_sb[0:1, :], start=True, stop=True,
                    )
                qk_proj_chunk(ps1a, "q", xq_b[0], wq_sb, bq_sb, qt_sb, 0, 0)
                for qc_ in range(4):
                    qk_proj_chunk(ps1a, "k", xk_b[0], wk_sb, bk_sb, kt_sb, 0, qc_)
                for qc_ in range(1, 4):
                    qk_proj_chunk(ps1a, "q", xq_b[0], wq_sb, bq_sb, qt_sb, 0, qc_)

            # batch-1 input DMAs, gated on the batch-0 projections via
            # marker copies (issuing all big loads at once overflows the
            # fast SWDGE ring)
            nc.vector.tensor_copy(xq_b[1][0:1, 0:1], qt_sb[0:1, 0:1])
            for qc_ in range(4):
                qk_chunk_dma(xq_b[1], xq, 1, qc_)
            nc.vector.tensor_copy(xk_b[1][0:1, 0:1], kt_sb[0:1, 0:1])
            for qc_ in range(4):
                qk_chunk_dma(xk_b[1], xk, 1, qc_)
            nc.vector.tensor_copy(xv_b[1][0:1, 0:1], kt_sb[0:1, 1537:1538])
            nc.sync.dma_start(xv_b[1][:], xv[:, W : 2 * W])

            # fc weights/residual aren't needed until ~190us -- gate their
            # DMAs on the batch-0 projections so they don't steal input
            # bandwidth at startup.  They must NOT sit on the scalar queue:
            # a gated dma_start there blocks the exp ACTIVATEs behind it
            # (strict FIFO).  GpSimd is idle until the attention finish path.
            nc.vector.tensor_copy(wf_sb[0:1, 0:1], kt_sb[0:1, 1537:1538])
            nc.gpsimd.dma_start(wf_sb[:], wf[:])
            nc.vector.tensor_copy(wf8_sb[0:1, 0:1], kt_sb[0:1, 1537:1538])
            nc.gpsimd.dma_start(wf8_sb[:], wf8[:])
            nc.vector.tensor_copy(xresfc_sb[0:1, 0:1], kt_sb[0:1, 1537:1538])
            nc.gpsimd.dma_start(xresfc_sb[:], xresfc[:])

            # =================== phase 2: attention ===================
            a2a_in = dram.tile([1024, 512], FP8E5)
            a2a_out = dram.tile([1024, 512], FP8E5)
            with (
                tc.tile_pool(name="stp", bufs=2, space="PSUM") as stp,  # 4 banks
                tc.tile_pool(name="ptp", bufs=10) as ptp,
                tc.tile_pool(name="nrm", bufs=3) as nrm,
            ):
                def emit_qk(qc, kt, pts):
                    # scores (two concurrent 64-row head tiles) + exp
                    b = qc // 4
                    q0 = 512 * qc
                    kk = 2048 * b + 128 * kt
                    st = stp.tile([128, 1024], F32, name=f"st{qc}_{kt}", tag="st")
                    if kt % 2 == 0:
                        pts.append(
                            ptp.tile([128, 2048], FP8, name=f"pt{qc}_{kt // 2}", tag="pt")
                        )
                    pt2 = pts[-1]
                    for h in range(2):
                        nc.tensor.matmul(
                            st[:, 512 * h : 512 * (h + 1)],
                            lhsT=kt_sb[64 * h : 64 * (h + 1), kk : kk + 128],
                            rhs=qt_sb[64 * h : 64 * (h + 1), q0 : q0 + 512],
                            start=True,
                            stop=True,
                        )
                    dst = pt2[:, 1024 * (kt % 2) : 1024 * (kt % 2 + 1)]
                    if _use_dve(qc, kt):
                        nc.vector.tensor_scalar(
                            dst.bitcast(U8), st[:], EXPA, EXPB, ALU.mult, ALU.add
                        )
                    else:
                        # bias=-3 rescales every exp by e^-3 (softmax-
                        # invariant) so the fp8 pt tiles can't overflow
                        nc.scalar.activation(
                            dst, st[:], AF.Exp, scale=EXP_SCALE, bias=nbias[:]
                        )

                def emit_pv(qc, pair, ots, pt2):
                    # fp8 DoubleRow over the k-tile pair.  lhsT includes the
                    # ones column, so row 64 of each OT accumulates the
                    # softmax denominator -- no separate rowsum matmuls.
                    b = qc // 4
                    vkt0 = 16 * b + 2 * pair
                    first = pair == 0
                    last = pair == NKT // 2 - 1
                    vr = v_sb[:].rearrange("p (t n) -> p t n", t=32)
                    pr = pt2[:].rearrange("p (two n) -> p two n", two=2)
                    for h in range(2):
                        nc.tensor.matmul(
                            ots[h][0:65, :],
                            lhsT=vr[:, vkt0 : vkt0 + 2, 65 * h : 65 * h + 65],
                            rhs=pr[:, :, 512 * h : 512 * (h + 1)],
                            start=first,
                            stop=last,
                            perf_mode=PM.DoubleRow,
                        )

                def finish(qc, ot_sb, rs_bf):
                    # broadcast the denominator row (partition 64) with K=1
                    # matmuls, approx-reciprocal on VectorE, multiply on
                    # GpSimd
                    q0 = 512 * qc
                    bc = ps1b.tile([128, 512], F32, name=f"bc{qc}", tag="ps")
                    nc.tensor.matmul(
                        bc[0:64, :], lhsT=ones_sb[64:65, 0:64],
                        rhs=rs_bf[64:65, 0:512],
                        start=True, stop=True, tile_position=(64, 0),
                    )
                    nc.tensor.matmul(
                        bc[64:128, :], lhsT=ones_sb[64:65, 0:64],
                        rhs=rs_bf[64:65, 512:1024],
                        start=True, stop=True, tile_position=(64, 64),
                    )
                    recipb = nrm.tile([128, 512], F32, name=f"recipb{qc}", tag="recipb")
                    nc.vector.reciprocal_approx_fast(recipb[:], bc[:])
                    nc.gpsimd.tensor_tensor(
                        zt_local[:, q0 : q0 + 512], ot_sb[:], recipb[:], ALU.mult
                    )
                    nc.sync.dma_start(
                        a2a_in[128 * qc : 128 * (qc + 1), :],
                        zt_local[:, q0 : q0 + 512],
                    )

                # deferred projection units interleaved into the attention
                # loop: batch-1 Q/K first, then V1
                with tc.tile_pool(name="ps1b", bufs=2, space="PSUM") as ps1b:
                    units = []
                    for qc_ in range(4):
                        units += qk_proj_subunits(
                            ps1b, "q", xq_b[1], wq_sb, bq_sb, qt_sb, 1, qc_
                        )
                    for qc_ in range(4):
                        units += qk_proj_subunits(
                            ps1b, "k", xk_b[1], wk_sb, bk_sb, kt_sb, 1, qc_
                        )
                    for kt in range(NKT):
                        units += v_subunits(ps1b, 1, kt)
                    sched = {}
                    for s, n in (
                        [((0, p), 2) for p in range(4, 8)]   # q1: 8 subs
                        + [((1, p), 2) for p in range(4, 8)]  # k1: 8 subs
                        + [((2, p), 2) for p in range(8)]     # v1 kt0-7
                        + [((3, p), 2) for p in range(8)]     # v1 kt8-15
                    ):
                        sched[s] = sched.get(s, 0) + n
                    ui = 0

                    # prologue: qk(qc0); V0 units interleaved only from kt4
                    # on, so the first scores aren't gated on the xv0 DMA
                    v0_units = []
                    for kt in range(NKT):
                        v0_units += v_subunits(ps1b, 0, kt)
                    v0i = 0
                    pts = []
                    for kt in range(NKT):
                        emit_qk(0, kt, pts)
                        if kt >= 4:
                            take = 3 if kt < 12 else 2
                            for _ in range(take):
                                if v0i < len(v0_units):
                                    v0_units[v0i]()
                                    v0i += 1
                    while v0i < len(v0_units):
                        v0_units[v0i]()
                        v0i += 1

                    otp = tc.alloc_tile_pool(name="otp", bufs=1, space="PSUM")
                    otp2 = tc.alloc_tile_pool(name="otp2", bufs=1, space="PSUM")
                    pending = None
                    for qc in range(NQC):
                        ots = [
                            otp.tile([65, 512], F32, name=f"ota{qc}", tag="ot"),
                            otp2.tile([65, 512], F32, name=f"otb{qc}", tag="ot2"),
                        ]
                        nxt = []
                        for pair in range(NKT // 2):
                            emit_pv(qc, pair, ots, pts[pair])
                            if qc + 1 < NQC:
                                emit_qk(qc + 1, 2 * pair, nxt)
                                emit_qk(qc + 1, 2 * pair + 1, nxt)
                            for _ in range(sched.get((qc, pair), 0)):
                                if ui < len(units):
                                    units[ui]()
                                    ui += 1
                            if pair == 1 and pending is not None:
                                finish(*pending)
                                pending = None
                        pts = nxt
                        # drain psum accumulators to SBUF on VectorE; the
                        # denominator rows (partition 64) go to bf16 for
                        # the broadcast matmuls
                        ot_sb = nrm.tile([128, 512], F32, name=f"otsb{qc}", tag="otsb")
                        nc.vector.tensor_copy(ot_sb[0:64, :], ots[0][0:64, :])
                        nc.vector.tensor_copy(ot_sb[64:128, :], ots[1][0:64, :])
                        rs_bf = nrm.tile([65, 1024], BF16, name=f"rsbf{qc}", tag="rsbf")
                        nc.vector.tensor_copy(rs_bf[64:65, 0:512], ots[0][64:65, :])
                        nc.vector.tensor_copy(rs_bf[64:65, 512:1024], ots[1][64:65, :])
                        if pending is not None:
                            finish(*pending)
                        pending = (qc, ot_sb, rs_bf)
                    while ui < len(units):  # safety: finish any leftovers
                        units[ui]()
                        ui += 1
                    finish(*pending)
                    otp2.release()
                    otp.release()

            xin.release()
            xvp.release()

            # ========== phase 3: fc (part 1) + A2A + fc (part 2) ==========
            with (
                tc.tile_pool(name="fcps", bufs=1, space="PSUM") as fcps,
                tc.tile_pool(name="ysb", bufs=2) as ysb,
            ):
                yps = {}
                for qt in range(4):
                    for nb in range(2):
                        yps[(qt, nb)] = fcps.tile(
                            [128, 512], F32, name=f"yp{qt}_{nb}", tag=f"yp{qt}_{nb}"
                        )

                def y1(qt, nb):
                    yp = yps[(qt, nb)]
                    for j in range(NDIN):
                        nc.tensor.matmul(
                            yp[:],
                            lhsT=xresfc_sb[:, 512 * j + 128 * qt : 512 * j + 128 * (qt + 1)],
                            rhs=wf_sb[:, 1024 * j + 512 * nb : 1024 * j + 512 * (nb + 1)],
                            start=(j == 0),
                            stop=False,
                        )

                # Y1 for 7 of 8 PSUM tiles, keep-warm dummies spanning the
                # collective, then the last Y1
                for qt in range(4):
                    for nb in range(2):
                        if (qt, nb) != (0, 0):
                            y1(qt, nb)
                for i in range(KEEPWARM_MM):
                    nc.tensor.matmul(
                        yps[(0, 0)][0:1, :],
                        lhsT=ones_sb[0:1, 0:1],
                        rhs=wf_sb[0:1, 0:512],
                        start=True,
                        stop=True,
                    )
                y1(0, 0)

                nc.gpsimd.collective_compute(
                    "AllToAll",
                    ALU.bypass,
                    replica_groups=[list(range(N_CORES))],
                    ins=[a2a_in.opt()],
                    outs=[a2a_out.opt()],
                )
                for t in range(NQC):
                    nc.sync.dma_start(
                        zt_local[:, 512 * t : 512 * (t + 1)],
                        a2a_out[128 * t : 128 * (t + 1), :],
                    )

                # Y2: u-outer so each slab-pair's 8 matmuls run as soon as
                # its A2A return DMAs land, instead of waiting for all 8
                zr = zt_local[:].rearrange("p (j v) -> p j v", j=NDIN)
                wr8 = wf8_sb[:].rearrange("p (j n) -> p j n", j=NDIN)
                for u in range(NDIN // 2):
                    for qt in range(4):
                        for nb in range(2):
                            nc.tensor.matmul(
                                yps[(qt, nb)][:],
                                lhsT=zr[:, 2 * u : 2 * u + 2, 128 * qt : 128 * (qt + 1)],
                                rhs=wr8[:, 2 * u : 2 * u + 2, 512 * nb : 512 * (nb + 1)],
                                start=False,
                                stop=(u == NDIN // 2 - 1),
                                perf_mode=PM.DoubleRow,
                            )
                for qt in range(4):
                    y = ysb.tile([128, 1024], F32, name=f"y{qt}", tag="y")
                    for nb in range(2):
                        nc.vector.tensor_copy(
                            y[:, 512 * nb : 512 * (nb + 1)], yps[(qt, nb)][:]
                        )
                        eng = nc.sync if (2 * qt + nb) % 2 == 0 else nc.scalar
                        eng.dma_start(
                            out[128 * qt : 128 * (qt + 1), 512 * nb : 512 * (nb + 1)],
                            y[:, 512 * nb : 512 * (nb + 1)],
                        )

    nc.compile()
    return nc


def _numpy_reference(q_, k_, v_, mask, Wq, bq, Wk, bk, Wv, bv, Wf, bf):
    q_ = np.asarray(q_, np.float32)
    k_ = np.asarray(k_, np.float32)
    v_ = np.asarray(v_, np.float32)
    b = q_.shape[0]

    def split(x):
        return x.reshape(b, -1, H, DH).transpose(0, 2, 1, 3)

    q = split(q_ @ Wq + bq)
    k = split(k_ @ Wk + bk)
    v = split(v_ @ Wv + bv)
    attn = np.einsum("bhqd,bhkd->bhqk", q, k) / np.sqrt(np.float32(DH))
    attn = np.where(np.asarray(mask)[:, None, :, None], attn, np.float32(-1e12))
    attn = attn - attn.max(axis=-1, keepdims=True)
    e = np.exp(attn)
    p = e / e.sum(axis=-1, keepdims=True)
    o = np.einsum("bhqk,bhkd->bhqd", p, v)
    o = o.transpose(0, 2, 1, 3).reshape(b, -1, D)
    return (o + q_) @ Wf + bf


def _tile_pmaj(x, width):
    # [D, N] -> [128, NDIN * N] with din-tile-major free dim
    d, n = x.shape
    t = d // 128
    return np.ascontiguousarray(
        x.reshape(t, 128, n).transpose(1, 0, 2).reshape(128, t * n)
    )


def kernel(q_, k_, v_, mask, Wq, bq, Wk, bk, Wv, bv, Wf, bf):
    mask = np.asarray(mask)
    if not mask.all():
        return _numpy_reference(q_, k_, v_, mask, Wq, bq, Wk, bk, Wv, bv, Wf, bf)

    q_ = np.asarray(q_, np.float32)
    k_ = np.asarray(k_, np.float32)
    v_ = np.asarray(v_, np.float32)

    def prep_qk(x):
        # [B,S,D] -> [128, (b qc t v)]: out[p, ((b*4+qc)*8+t)*512+v] =
        # x[b, 512*qc+v, 128*t+p]
        r = x.transpose(2, 0, 1).reshape(NDIN, 128, B, 4, 512).transpose(1, 2, 3, 0, 4)
        return np.ascontiguousarray(r.reshape(128, B * 4 * QCW)).astype(FP8NP)

    def prep_xv(x):
        # [B,S,D] -> [128, (b t v)] with v=2048
        xt = np.concatenate([x[b].T for b in range(B)], axis=1)  # [D, B*S]
        r = xt.reshape(NDIN, 128, B, S).transpose(1, 2, 0, 3)
        return np.ascontiguousarray(r.reshape(128, B * NDIN * S)).astype(FP8NP)

    xqh = prep_qk(q_)
    xkh = prep_qk(k_)
    xvh = prep_xv(v_)
    Wf32 = np.asarray(Wf, np.float32)
    wf_b = _tile_pmaj(Wf32 / WSCALE, 1024).astype(BF16NP)
    wf8_b = _tile_pmaj(Wf32 / WSCALE, 1024).astype(FP8E5NP)

    in_maps = []
    for c in range(N_CORES):
        d0 = 128 * c
        # wv per-core: [128, NDIN, 130]: [h0 64 | 0 | h1 64 | 0] per tile
        wvs = np.asarray(Wv, np.float32)[:, d0 : d0 + 128] * WSCALE
        wv130 = np.zeros((NDIN, 128, VTW), np.float32)
        wvt = wvs.reshape(NDIN, 128, 128)
        wv130[:, :, 0:64] = wvt[:, :, 0:64]
        wv130[:, :, 65:129] = wvt[:, :, 64:128]
        wv130 = np.ascontiguousarray(
            wv130.transpose(1, 0, 2).reshape(128, NDIN * VTW)
        ).astype(FP8NP)
        bvc = np.asarray(bv, np.float32)[d0 : d0 + 128] * WSCALE
        bvx = np.zeros((1, VTW), np.float32)
        bvx[0, 0:64] = bvc[0:64]
        bvx[0, 64] = 1.0
        bvx[0, 65:129] = bvc[64:128]
        bvx[0, 129] = 1.0
        in_maps.append(
            {
                "xq": xqh,
                "xk": xkh,
                "xv": xvh,
                "xresfc": _tile_pmaj(
                    np.ascontiguousarray(
                        q_[c // 4].T[:, 512 * (c % 4) : 512 * (c % 4 + 1)] * WSCALE
                    ),
                    512,
                ).astype(BF16NP),
                "wq": _tile_pmaj(
                    np.asarray(Wq, np.float32)[:, d0 : d0 + 128] * WSCALE, 128
                ).astype(FP8NP),
                "wk": _tile_pmaj(
                    np.asarray(Wk, np.float32)[:, d0 : d0 + 128] * WSCALE, 128
                ).astype(FP8NP),
                "wv": wv130,
                "wf": wf_b,
                "wf8": wf8_b,
                "bq": np.ascontiguousarray(np.asarray(bq, np.float32)[d0 : d0 + 128, None] * WSCALE),
                "bk": np.ascontiguousarray(np.asarray(bk, np.float32)[d0 : d0 + 128, None] * WSCALE),
                "bvx": bvx.astype(BF16NP),
            }
        )

    if "nc" not in _CACHE:
        _CACHE["nc"] = _build()
    res = run_bass_kernel_spmd(_CACHE["nc"], in_maps, core_ids=list(range(N_CORES)))

    out = np.empty((B, S, D), np.float32)
    for c in range(N_CORES):
        y = res.results[c]["out"]
        out[c // 4, 512 * (c % 4) : 512 * (c % 4 + 1), :] = y
    out += np.asarray(bf, np.float32)[None, None, :]
    return out


if __name__ == "__main__":
    rng = np.random.default_rng(0)
    args = dict(
        q_=rng.standard_normal((B, S, D), dtype=np.float32),
        k_=rng.standard_normal((B, S, D), dtype=np.float32),
        v_=rng.standard_normal((B, S, D), dtype=np.float32),
        mask=np.ones((B, S), bool),
        Wq=rng.standard_normal((D, D), dtype=np.float32) * 0.02,
        bq=np.zeros(D, np.float32),
        Wk=rng.standard_normal((D, D), dtype=np.float32) * 0.02,
        bk=np.zeros(D, np.float32),
        Wv=rng.standard_normal((D, D), dtype=np.float32) * 0.02,
        bv=np.zeros(D, np.float32),
        Wf=rng.standard_normal((D, D), dtype=np.float32) * 0.02,
        bf=np.zeros(D, np.float32),
    )
    got = kernel(**args)
    want = _numpy_reference(**args)
    rel = np.abs(got - want).max() / np.abs(want).max()
    print("rel_err:", rel)
